# revision 50
# baseline (speedup 1.0000x reference)
"""AdaptiveCurvatureLoss on 8 TRN2 NeuronCores — bucketed exact kNN via
matmul + DVE top-8, no sort.

The reference needs, per element, the mean of the 3 smallest |x_i - x_j|
(incl. self-zero), then only mean(dens) and max(dens).  Host shards the
N=16384 samples by VALUE into 128 quantile buckets of exactly 128 elements
(one np.partition call — the sharding step), 16 buckets per core.  The two
nearest neighbours of any element provably lie inside its bucket plus a
2-element halo on each side, so each core evaluates the reference's NxN
pairwise matrix restricted to 16 row-blocks of [128 x 132]:

  -d^2[i,j] = -(x_i - x_j)^2  via one K=10 bf16 matmul per block
              (two-limb bf16 split of x and three-limb split of x^2 keeps
              products exact in fp32 PSUM; -d^2 error ~1e-10, self == ~0),
  top-3 nearest = DVE max8 (top-8 per partition) straight out of PSUM.

The MLP / second-derivative / MSE parts run in the transposed layout
(hidden units on partitions, 512 points per tile, two 64-wide h-blocks per
128 partitions): u = w1*x + b1 as one K=6 two-limb bf16 matmul, tanh /
square on ACT, g = (1-th^2)*th on DVE, then bf16 PE matmuls compute
e = pred + b2 - t and d2 in a single [8,512] PSUM tile; one ACT
Square+accum yields the 8 scalar partials.

Host epilogue (O(N) numpy): d1+d2 = sqrt of the top-2 non-self -d^2,
density mean/max, final three scalars.
"""

import sys

sys.path.insert(0, "/opt/trn_rl_repo")

import numpy as np

import concourse.mybir as mybir
from concourse import bacc
from concourse.bass_utils import run_bass_kernel_spmd
from concourse.tile import TileContext

N = 16384
NCORES = 8
SHARD = N // NCORES          # 2048
P = 128
NB = N // P                  # 128 value buckets of 128 elements
TPC = NB // NCORES           # 16 buckets (tiles) per core
C = P + 4                    # candidate columns: bucket + 2-elem halo each side
KK = 10                      # K rows of the kNN matmul
EPS = 1e-8
SENT = 1e8                   # sentinel -d^2 magnitude for missing halo
F32 = mybir.dt.float32
BF16 = mybir.dt.bfloat16
ALU = mybir.AluOpType
ACTF = mybir.ActivationFunctionType

NT = 2                       # packed [128, 512] MLP tiles per core (2048 pts)


def _build():
    nc = bacc.Bacc()
    klb = nc.declare_dram_parameter("klb", [KK, TPC * P], BF16, isOutput=False)
    krb = nc.declare_dram_parameter("krb", [KK, TPC * C], BF16, isOutput=False)
    xmw = nc.declare_dram_parameter("xmw", [6, NT * 512 + P], BF16, isOutput=False)
    wpc = nc.declare_dram_parameter("wpc", [P, 4 * 8], BF16, isOutput=False)
    td = nc.declare_dram_parameter("td", [P, TPC * 8], F32, isOutput=True)
    po = nc.declare_dram_parameter("po", [8, 512], F32, isOutput=True)

    with TileContext(nc) as tc:
        with (
            tc.tile_pool(name="sp", bufs=1) as sp,
            tc.tile_pool(name="kp", bufs=4, space="PSUM") as kp,
            tc.tile_pool(name="mu", bufs=2, space="PSUM") as mu,
            tc.tile_pool(name="ms", bufs=2, space="PSUM") as ms,
        ):
            # loads: xmw first on sync (heads the serial MLP chain), kNN
            # packs split so tile-0 inputs land earliest
            HP, HC = (TPC // 2) * P, (TPC // 2) * C
            klbt = sp.tile([KK, TPC * P], BF16)
            krbt = sp.tile([KK, TPC * C], BF16)
            xmt = sp.tile([6, NT * 512 + P], BF16)
            nc.sync.dma_start(xmt[:, :], xmw[:, :])
            nc.scalar.dma_start(klbt[:, 0:HP], klb[:, 0:HP])
            nc.sync.dma_start(krbt[:, 0:HC], krb[:, 0:HC])
            nc.scalar.dma_start(klbt[:, HP:], klb[:, HP:])
            nc.sync.dma_start(krbt[:, HC:], krb[:, HC:])
            wpt = sp.tile([P, 4 * 8], BF16)
            nc.gpsimd.dma_start(wpt[:, :], wpc[:, :])

            th = sp.tile([P, NT, 512], BF16)
            th2 = sp.tile([P, NT, 512], BF16)
            th3 = sp.tile([P, NT, 512], BF16)
            tds = sp.tile([P, TPC * 8], F32)
            posb = sp.tile([8, 512], F32)

            ps8 = ms.tile([8, 512], F32, tag="ms")

            # ---- emitters ----
            def emit_knn(t):
                ps = kp.tile([P, C], F32, tag="kp")
                nc.tensor.matmul(
                    ps[:, :], klbt[:, t * P : (t + 1) * P],
                    krbt[:, t * C : (t + 1) * C], start=True, stop=True,
                )
                nc.vector.max(tds[:, 8 * t : 8 * t + 8], ps[:, :])

            def emit_u_tanh(t):
                ups = mu.tile([P, 512], F32, tag="mu")
                nc.tensor.matmul(
                    ups[:, :], xmt[:, NT * 512 : NT * 512 + P],
                    xmt[:, t * 512 : (t + 1) * 512], start=True, stop=True,
                )
                nc.scalar.activation(th[:, t, :], ups[:, :], ACTF.Tanh)

            def emit_sq_all():
                nc.scalar.activation(
                    th2[:, :, :].rearrange("p a b -> p (a b)"),
                    th[:, :, :].rearrange("p a b -> p (a b)"), ACTF.Square,
                )

            def emit_th3_all():
                # th^3 = th2 * th: plain bf16 TT -> DVE 2x_1P, one op both tiles
                nc.vector.tensor_tensor(
                    th3[:, :, :].rearrange("p a b -> p (a b)"),
                    th2[:, :, :].rearrange("p a b -> p (a b)"),
                    th[:, :, :].rearrange("p a b -> p (a b)"),
                    op=ALU.mult,
                )

            def emit_mlp_a(t):
                # one matmul: psum rows 0:4 get pred (w2), rows 4:8 get
                # -c2n*th, via an 8-col zero-padded lhsT
                nc.tensor.matmul(
                    ps8[:, :], wpt[:, 8 * t : 8 * t + 8], th[:, t, :],
                    start=(t == 0), stop=False,
                )

            def emit_mlp_b(t):
                # rows 4:8 += c2n*th3 (cols 0:4 of the pack are zero)
                nc.tensor.matmul(
                    ps8[:, :], wpt[:, 16 + 8 * t : 24 + 8 * t], th3[:, t, :],
                    start=False, stop=(t == NT - 1),
                )

            def emit_po_out():
                # pred and d2 out; host does targets/squares
                nc.scalar.activation(posb[:, :], ps8[:, :], ACTF.Copy)
                nc.gpsimd.dma_start(po[:, :], posb[:, :])

            # ---- schedule: MLP chain heads the queues (high priority so the
            # scheduler never parks its ops behind the kNN stream) ----
            with tc.high_priority():
                emit_u_tanh(0)
                emit_u_tanh(1)
            for t in range(TPC):
                emit_knn(t)
                if t == 0:
                    with tc.high_priority():
                        emit_sq_all()
                elif t == 1:
                    with tc.high_priority():
                        emit_mlp_a(0)
                        emit_mlp_a(1)
                elif t == 2:
                    with tc.high_priority():
                        emit_th3_all()
                elif t == 3:
                    with tc.high_priority():
                        emit_mlp_b(0)
                        emit_mlp_b(1)
                elif t == 4:
                    with tc.high_priority():
                        emit_po_out()
                if t % 4 == 3 and t < TPC - 1:
                    q = nc.sync if (t // 4) % 2 == 0 else nc.gpsimd
                    q.dma_start(
                        td[:, 8 * (t - 3) : 8 * (t + 1)],
                        tds[:, 8 * (t - 3) : 8 * (t + 1)],
                    )
            nc.gpsimd.dma_start(td[:, 96:120], tds[:, 96:120])
            nc.sync.dma_start(td[:, 120:128], tds[:, 120:128])
    nc.finalize()
    return nc


_NC_CACHE = None


def _get_nc():
    global _NC_CACHE
    if _NC_CACHE is None:
        _NC_CACHE = _build()
    return _NC_CACHE


def _b16(a):
    import ml_dtypes

    return np.asarray(a, dtype=np.float64).astype(ml_dtypes.bfloat16)


def _limbs2(v):
    """Split f64 array into 2 bf16 limbs (value approx h+l)."""
    h = _b16(v)
    l = _b16(np.asarray(v, np.float64) - h.astype(np.float64))
    return h, l


def _limbs3(v):
    h = _b16(v)
    r = np.asarray(v, np.float64) - h.astype(np.float64)
    m = _b16(r)
    lo = _b16(r - m.astype(np.float64))
    return h, m, lo


def make_in_maps(x_input, targets, w1, b1, w2, b2):
    import ml_dtypes

    x_input = np.ascontiguousarray(x_input, dtype=np.float32)
    targets = np.ascontiguousarray(targets, dtype=np.float32)
    w1 = np.asarray(w1, dtype=np.float32)
    b1 = np.asarray(b1, dtype=np.float32)
    w2 = np.asarray(w2, dtype=np.float32)
    b2 = np.asarray(b2, dtype=np.float32)

    # ---- value-bucket sharding: 128 buckets of exactly 128 + halo stats ----
    kth = np.unique(
        np.concatenate(
            [np.arange(1, NB) * P + d for d in (-2, -1, 0, 1)]
        )
    )
    part = np.partition(x_input, kth).astype(np.float64)

    klb_all = []
    krb_all = []
    for c in range(NCORES):
        klc = np.zeros((KK, TPC * P), np.float64)
        krc = np.zeros((KK, TPC * C), np.float64)
        krc[0:3, :] = -1.0
        for ti in range(TPC):
            b = c * TPC + ti
            blk = part[b * P : (b + 1) * P]
            lo = part[b * P - 2 : b * P] if b > 0 else None
            hi = part[(b + 1) * P : (b + 1) * P + 2] if b < NB - 1 else None
            ctr = np.float32((blk.min() + blk.max()) / 2.0)

            xi = (blk - ctr).astype(np.float32).astype(np.float64)
            xih, xil = _limbs2(xi)
            xi_hat = xih.astype(np.float64) + xil.astype(np.float64)
            si = xi_hat**2
            sih, sim, sil = _limbs3(si)
            one = np.ones(P)
            klc[:, ti * P : (ti + 1) * P] = np.stack(
                [
                    sih.astype(np.float64), sim.astype(np.float64),
                    sil.astype(np.float64),
                    xih.astype(np.float64), xih.astype(np.float64),
                    xil.astype(np.float64), xil.astype(np.float64),
                    one, one, one,
                ]
            )

            # candidates: [lo2 | bucket | hi2]
            cvals = np.zeros(C, np.float64)
            creal = np.ones(C, bool)
            cvals[2 : 2 + P] = blk
            if lo is not None:
                cvals[0:2] = lo
            else:
                creal[0:2] = False
            if hi is not None:
                cvals[2 + P :] = hi
            else:
                creal[2 + P :] = False
            xj = (cvals - ctr).astype(np.float32).astype(np.float64)
            xjh, xjl = _limbs2(xj)
            xj_hat = xjh.astype(np.float64) + xjl.astype(np.float64)
            sj = xj_hat**2
            sjh, sjm, sjl = _limbs3(sj)
            xjh64 = xjh.astype(np.float64)
            xjl64 = xjl.astype(np.float64)
            kr = np.stack(
                [
                    2 * xjh64, 2 * xjl64, 2 * xjh64, 2 * xjl64,
                    -sjh.astype(np.float64), -sjm.astype(np.float64),
                    -sjl.astype(np.float64),
                ]
            )
            # sentinel columns: x-limbs 0, s_hi = SENT -> -d^2 ~= -SENT
            bad = ~creal
            kr[0:4, bad] = 0.0
            kr[4, bad] = -SENT
            kr[5:7, bad] = 0.0
            krc[3:10, ti * C : (ti + 1) * C] = kr
        klb_all.append(klc)
        krb_all.append(krc)

    # ---- MLP packs (two-limb u-matmul, rest as in the sort baseline) ----
    w1h, w1l = _limbs2(w1)
    b1h, b1l = _limbs2(b1)
    H = 64
    wu = np.zeros((6, P), np.float64)
    wu[0, :H] = w1h.astype(np.float64)
    wu[1, :H] = w1l.astype(np.float64)
    wu[2, H:] = w1h.astype(np.float64)
    wu[3, H:] = w1l.astype(np.float64)
    wu[4, :H] = b1h.astype(np.float64)
    wu[4, H:] = b1h.astype(np.float64)
    wu[5, :H] = b1l.astype(np.float64)
    wu[5, H:] = b1l.astype(np.float64)

    c2n = (2.0 * w1.astype(np.float64) ** 2 * w2.astype(np.float64)).astype(
        np.float32
    )
    # 8-col zero-padded lhsT packs: A(t) = [pred | -c2n*th], B(t) = [0 | +c2n*th3]
    wp = np.zeros((P, 32), np.float32)
    for t in range(NT):
        a0 = 8 * t
        wp[:H, a0 + 2 * t] = w2
        wp[H:, a0 + 2 * t + 1] = w2
        wp[:H, a0 + 4 + 2 * t] = -c2n
        wp[H:, a0 + 4 + 2 * t + 1] = -c2n
        b0 = 16 + 8 * t
        wp[:H, b0 + 4 + 2 * t] = c2n
        wp[H:, b0 + 4 + 2 * t + 1] = c2n
    wp = wp.astype(ml_dtypes.bfloat16)

    in_maps = []
    for c in range(NCORES):
        xsh = x_input[c * SHARD : (c + 1) * SHARD].astype(np.float64)
        xm = np.zeros((6, NT * 512 + P), np.float64)
        for t in range(NT):
            xa = xsh[t * 1024 : t * 1024 + 512]
            xb = xsh[t * 1024 + 512 : (t + 1) * 1024]
            xah, xal = _limbs2(xa)
            xbh, xbl = _limbs2(xb)
            xm[0, t * 512 : (t + 1) * 512] = xah.astype(np.float64)
            xm[1, t * 512 : (t + 1) * 512] = xal.astype(np.float64)
            xm[2, t * 512 : (t + 1) * 512] = xbh.astype(np.float64)
            xm[3, t * 512 : (t + 1) * 512] = xbl.astype(np.float64)
        xm[4, : NT * 512] = 1.0
        xm[5, : NT * 512] = 0.0
        xm[:, NT * 512 :] = wu
        in_maps.append(
            {
                "klb": np.ascontiguousarray(klb_all[c].astype(ml_dtypes.bfloat16)),
                "krb": np.ascontiguousarray(krb_all[c].astype(ml_dtypes.bfloat16)),
                "xmw": np.ascontiguousarray(xm.astype(ml_dtypes.bfloat16)),
                "wpc": np.ascontiguousarray(wp),
            }
        )
    return in_maps


def kernel(x_input, targets, w1, b1, w2, b2, **_ignored):
    in_maps = make_in_maps(x_input, targets, w1, b1, w2, b2)
    nc = _get_nc()
    res = run_bass_kernel_spmd(nc, in_maps, core_ids=list(range(NCORES)))

    # ---- host epilogue: density from per-element top-2 non-self -d^2 ----
    targets64 = np.asarray(targets, np.float64)
    b20 = float(np.asarray(b2).ravel()[0])
    dsum = []
    sse = 0.0
    d2sq = 0.0
    for c, r in enumerate(res.results):
        t = r["td"].astype(np.float64).reshape(P, TPC, 8)
        tt = t[:, :, 1:3]  # 2nd/3rd largest = the two nearest (non-self)
        d = np.sqrt(np.maximum(-tt, 0.0))
        dsum.append(d.sum(axis=2).ravel())
        pr = r["po"].astype(np.float64)  # rows 0:4 pred, 4:8 d2
        tsh = targets64[c * SHARD : (c + 1) * SHARD]
        for tt_ in range(NT):
            e_a = pr[2 * tt_] + b20 - tsh[tt_ * 1024 : tt_ * 1024 + 512]
            e_b = pr[2 * tt_ + 1] + b20 - tsh[tt_ * 1024 + 512 : (tt_ + 1) * 1024]
            sse += (e_a**2).sum() + (e_b**2).sum()
        d2sq += (pr[4:8] ** 2).sum()
    d12 = np.concatenate(dsum)
    dens = 1.0 / (d12 / 3.0 + 2.0 * EPS)
    m = (dens.sum() / N) / (dens.max() + EPS)

    mse = sse / N
    penalty = 0.01 * (1.0 + 0.1 * m) * (d2sq / N)
    total = mse + penalty
    return np.array([total, mse, penalty], dtype=np.float32)


# revision 52
# speedup vs baseline: 1.1513x; 1.1513x over previous
"""AdaptiveCurvatureLoss on 8 TRN2 NeuronCores — bucketed exact kNN via
matmul + DVE top-8, no sort.

The reference needs, per element, the mean of the 3 smallest |x_i - x_j|
(incl. self-zero), then only mean(dens) and max(dens).  Host shards the
N=16384 samples by VALUE into 128 quantile buckets of exactly 128 elements
(one np.partition call — the sharding step), 16 buckets per core.  The two
nearest neighbours of any element provably lie inside its bucket plus a
2-element halo on each side, so each core evaluates the reference's NxN
pairwise matrix restricted to 16 row-blocks of [128 x 132]:

  -d^2[i,j] = -(x_i - x_j)^2  via one K=10 bf16 matmul per block
              (two-limb bf16 split of x and three-limb split of x^2 keeps
              products exact in fp32 PSUM; -d^2 error ~1e-10, self == ~0),
  top-3 nearest = DVE max8 (top-8 per partition) straight out of PSUM.

The MLP / second-derivative / MSE parts run in the transposed layout
(hidden units on partitions, 512 points per tile, two 64-wide h-blocks per
128 partitions): u = w1*x + b1 as one K=6 two-limb bf16 matmul, tanh /
square on ACT, g = (1-th^2)*th on DVE, then bf16 PE matmuls compute
e = pred + b2 - t and d2 in a single [8,512] PSUM tile; one ACT
Square+accum yields the 8 scalar partials.

Host epilogue (O(N) numpy): d1+d2 = sqrt of the top-2 non-self -d^2,
density mean/max, final three scalars.
"""

import sys

sys.path.insert(0, "/opt/trn_rl_repo")

import numpy as np

import concourse.mybir as mybir
from concourse import bacc
from concourse.bass_utils import run_bass_kernel_spmd
from concourse.tile import TileContext

N = 16384
NCORES = 8
SHARD = N // NCORES          # 2048
P = 128
NB = N // P                  # 128 value buckets of 128 elements
TPC = NB // NCORES           # 16 buckets (tiles) per core
C = P + 4                    # candidate columns: bucket + 2-elem halo each side
KK = 10                      # K rows of the kNN matmul
EPS = 1e-8
SENT = 1e8                   # sentinel -d^2 magnitude for missing halo
F32 = mybir.dt.float32
BF16 = mybir.dt.bfloat16
ALU = mybir.AluOpType
ACTF = mybir.ActivationFunctionType

NT = 2                       # packed [128, 512] MLP tiles per core (2048 pts)


def _build():
    nc = bacc.Bacc()
    klb = nc.declare_dram_parameter("klb", [KK, TPC * P], BF16, isOutput=False)
    krb = nc.declare_dram_parameter("krb", [KK, TPC * C], BF16, isOutput=False)
    xmw = nc.declare_dram_parameter("xmw", [6, NT * 512 + P], BF16, isOutput=False)
    wpc = nc.declare_dram_parameter("wpc", [P, 4 * 8], BF16, isOutput=False)
    td = nc.declare_dram_parameter("td", [P, TPC * 8], F32, isOutput=True)
    po = nc.declare_dram_parameter("po", [8, 512], F32, isOutput=True)

    with TileContext(nc) as tc:
        with (
            tc.tile_pool(name="sp", bufs=1) as sp,
            tc.tile_pool(name="kp", bufs=4, space="PSUM") as kp,
            tc.tile_pool(name="mu", bufs=2, space="PSUM") as mu,
            tc.tile_pool(name="ms", bufs=2, space="PSUM") as ms,
        ):
            # loads: xmw first on sync (heads the serial MLP chain), kNN
            # packs split so tile-0 inputs land earliest
            HP, HC = (TPC // 2) * P, (TPC // 2) * C
            klbt = sp.tile([KK, TPC * P], BF16)
            krbt = sp.tile([KK, TPC * C], BF16)
            xmt = sp.tile([6, NT * 512 + P], BF16)
            nc.gpsimd.dma_start(xmt[:, :], xmw[:, :])
            nc.sync.dma_start(krbt[:, 0:HC], krb[:, 0:HC])
            nc.scalar.dma_start(klbt[:, 0:HP], klb[:, 0:HP])
            nc.sync.dma_start(krbt[:, HC:], krb[:, HC:])
            nc.scalar.dma_start(klbt[:, HP:], klb[:, HP:])
            wpt = sp.tile([P, 4 * 8], BF16)
            nc.gpsimd.dma_start(wpt[:, :], wpc[:, :])

            th = sp.tile([P, NT, 512], BF16)
            th2 = sp.tile([P, NT, 512], BF16)
            th3 = sp.tile([P, NT, 512], BF16)
            tds = sp.tile([P, TPC * 8], F32)
            posb = sp.tile([8, 512], F32)

            ps8 = ms.tile([8, 512], F32, tag="ms")

            # ---- emitters ----
            def emit_knn(t):
                ps = kp.tile([P, C], F32, tag="kp")
                nc.tensor.matmul(
                    ps[:, :], klbt[:, t * P : (t + 1) * P],
                    krbt[:, t * C : (t + 1) * C], start=True, stop=True,
                )
                # bias the scheduler: let the MLP's th3 TT slot in before the
                # late max8s instead of after all of them
                with tc.tile_wait_until(0.013, enable=(t >= 8)):
                    nc.vector.max(tds[:, 8 * t : 8 * t + 8], ps[:, :])

            def emit_u_tanh(t):
                ups = mu.tile([P, 512], F32, tag="mu")
                nc.tensor.matmul(
                    ups[:, :], xmt[:, NT * 512 : NT * 512 + P],
                    xmt[:, t * 512 : (t + 1) * 512], start=True, stop=True,
                )
                nc.scalar.activation(th[:, t, :], ups[:, :], ACTF.Tanh)

            def emit_sq_all():
                nc.scalar.activation(
                    th2[:, :, :].rearrange("p a b -> p (a b)"),
                    th[:, :, :].rearrange("p a b -> p (a b)"), ACTF.Square,
                )

            def emit_th3_all():
                # th^3 = th2 * th: plain bf16 TT -> DVE 2x_1P, one op both tiles
                nc.vector.tensor_tensor(
                    th3[:, :, :].rearrange("p a b -> p (a b)"),
                    th2[:, :, :].rearrange("p a b -> p (a b)"),
                    th[:, :, :].rearrange("p a b -> p (a b)"),
                    op=ALU.mult,
                )

            def emit_mlp_a(t):
                # one matmul: psum rows 0:4 get pred (w2), rows 4:8 get
                # -c2n*th, via an 8-col zero-padded lhsT
                nc.tensor.matmul(
                    ps8[:, :], wpt[:, 8 * t : 8 * t + 8], th[:, t, :],
                    start=(t == 0), stop=False,
                )

            def emit_mlp_b(t):
                # rows 4:8 += c2n*th3 (cols 0:4 of the pack are zero)
                nc.tensor.matmul(
                    ps8[:, :], wpt[:, 16 + 8 * t : 24 + 8 * t], th3[:, t, :],
                    start=False, stop=(t == NT - 1),
                )

            def emit_po_out():
                # pred and d2 out; host does targets/squares
                nc.scalar.activation(posb[:, :], ps8[:, :], ACTF.Copy)
                nc.gpsimd.dma_start(po[:, :], posb[:, :])

            # ---- schedule: MLP chain heads the queues (high priority so the
            # scheduler never parks its ops behind the kNN stream) ----
            with tc.high_priority():
                emit_u_tanh(0)
                emit_u_tanh(1)
            for t in range(TPC):
                emit_knn(t)
                if t == 0:
                    with tc.high_priority():
                        emit_sq_all()
                elif t == 1:
                    with tc.high_priority():
                        emit_mlp_a(0)
                        emit_mlp_a(1)
                elif t == 2:
                    with tc.high_priority():
                        emit_th3_all()
                elif t == 3:
                    with tc.high_priority():
                        emit_mlp_b(0)
                        emit_mlp_b(1)
                elif t == 4:
                    with tc.high_priority():
                        emit_po_out()
                if t % 4 == 3 and t < TPC - 1:
                    q = nc.sync if (t // 4) % 2 == 0 else nc.gpsimd
                    q.dma_start(
                        td[:, 8 * (t - 3) : 8 * (t + 1)],
                        tds[:, 8 * (t - 3) : 8 * (t + 1)],
                    )
            nc.gpsimd.dma_start(td[:, 96:120], tds[:, 96:120])
            nc.sync.dma_start(td[:, 120:128], tds[:, 120:128])
    nc.finalize()
    return nc


_NC_CACHE = None


def _get_nc():
    global _NC_CACHE
    if _NC_CACHE is None:
        _NC_CACHE = _build()
    return _NC_CACHE


def _b16(a):
    import ml_dtypes

    return np.asarray(a, dtype=np.float64).astype(ml_dtypes.bfloat16)


def _limbs2(v):
    """Split f64 array into 2 bf16 limbs (value approx h+l)."""
    h = _b16(v)
    l = _b16(np.asarray(v, np.float64) - h.astype(np.float64))
    return h, l


def _limbs3(v):
    h = _b16(v)
    r = np.asarray(v, np.float64) - h.astype(np.float64)
    m = _b16(r)
    lo = _b16(r - m.astype(np.float64))
    return h, m, lo


def make_in_maps(x_input, targets, w1, b1, w2, b2):
    import ml_dtypes

    x_input = np.ascontiguousarray(x_input, dtype=np.float32)
    targets = np.ascontiguousarray(targets, dtype=np.float32)
    w1 = np.asarray(w1, dtype=np.float32)
    b1 = np.asarray(b1, dtype=np.float32)
    w2 = np.asarray(w2, dtype=np.float32)
    b2 = np.asarray(b2, dtype=np.float32)

    # ---- value-bucket sharding: 128 buckets of exactly 128 + halo stats ----
    kth = np.unique(
        np.concatenate(
            [np.arange(1, NB) * P + d for d in (-2, -1, 0, 1)]
        )
    )
    part = np.partition(x_input, kth).astype(np.float64)

    klb_all = []
    krb_all = []
    for c in range(NCORES):
        klc = np.zeros((KK, TPC * P), np.float64)
        krc = np.zeros((KK, TPC * C), np.float64)
        krc[0:3, :] = -1.0
        for ti in range(TPC):
            b = c * TPC + ti
            blk = part[b * P : (b + 1) * P]
            lo = part[b * P - 2 : b * P] if b > 0 else None
            hi = part[(b + 1) * P : (b + 1) * P + 2] if b < NB - 1 else None
            ctr = np.float32((blk.min() + blk.max()) / 2.0)

            xi = (blk - ctr).astype(np.float32).astype(np.float64)
            xih, xil = _limbs2(xi)
            xi_hat = xih.astype(np.float64) + xil.astype(np.float64)
            si = xi_hat**2
            sih, sim, sil = _limbs3(si)
            one = np.ones(P)
            klc[:, ti * P : (ti + 1) * P] = np.stack(
                [
                    sih.astype(np.float64), sim.astype(np.float64),
                    sil.astype(np.float64),
                    xih.astype(np.float64), xih.astype(np.float64),
                    xil.astype(np.float64), xil.astype(np.float64),
                    one, one, one,
                ]
            )

            # candidates: [lo2 | bucket | hi2]
            cvals = np.zeros(C, np.float64)
            creal = np.ones(C, bool)
            cvals[2 : 2 + P] = blk
            if lo is not None:
                cvals[0:2] = lo
            else:
                creal[0:2] = False
            if hi is not None:
                cvals[2 + P :] = hi
            else:
                creal[2 + P :] = False
            xj = (cvals - ctr).astype(np.float32).astype(np.float64)
            xjh, xjl = _limbs2(xj)
            xj_hat = xjh.astype(np.float64) + xjl.astype(np.float64)
            sj = xj_hat**2
            sjh, sjm, sjl = _limbs3(sj)
            xjh64 = xjh.astype(np.float64)
            xjl64 = xjl.astype(np.float64)
            kr = np.stack(
                [
                    2 * xjh64, 2 * xjl64, 2 * xjh64, 2 * xjl64,
                    -sjh.astype(np.float64), -sjm.astype(np.float64),
                    -sjl.astype(np.float64),
                ]
            )
            # sentinel columns: x-limbs 0, s_hi = SENT -> -d^2 ~= -SENT
            bad = ~creal
            kr[0:4, bad] = 0.0
            kr[4, bad] = -SENT
            kr[5:7, bad] = 0.0
            krc[3:10, ti * C : (ti + 1) * C] = kr
        klb_all.append(klc)
        krb_all.append(krc)

    # ---- MLP packs (two-limb u-matmul, rest as in the sort baseline) ----
    w1h, w1l = _limbs2(w1)
    b1h, b1l = _limbs2(b1)
    H = 64
    wu = np.zeros((6, P), np.float64)
    wu[0, :H] = w1h.astype(np.float64)
    wu[1, :H] = w1l.astype(np.float64)
    wu[2, H:] = w1h.astype(np.float64)
    wu[3, H:] = w1l.astype(np.float64)
    wu[4, :H] = b1h.astype(np.float64)
    wu[4, H:] = b1h.astype(np.float64)
    wu[5, :H] = b1l.astype(np.float64)
    wu[5, H:] = b1l.astype(np.float64)

    c2n = (2.0 * w1.astype(np.float64) ** 2 * w2.astype(np.float64)).astype(
        np.float32
    )
    # 8-col zero-padded lhsT packs: A(t) = [pred | -c2n*th], B(t) = [0 | +c2n*th3]
    wp = np.zeros((P, 32), np.float32)
    for t in range(NT):
        a0 = 8 * t
        wp[:H, a0 + 2 * t] = w2
        wp[H:, a0 + 2 * t + 1] = w2
        wp[:H, a0 + 4 + 2 * t] = -c2n
        wp[H:, a0 + 4 + 2 * t + 1] = -c2n
        b0 = 16 + 8 * t
        wp[:H, b0 + 4 + 2 * t] = c2n
        wp[H:, b0 + 4 + 2 * t + 1] = c2n
    wp = wp.astype(ml_dtypes.bfloat16)

    in_maps = []
    for c in range(NCORES):
        xsh = x_input[c * SHARD : (c + 1) * SHARD].astype(np.float64)
        xm = np.zeros((6, NT * 512 + P), np.float64)
        for t in range(NT):
            xa = xsh[t * 1024 : t * 1024 + 512]
            xb = xsh[t * 1024 + 512 : (t + 1) * 1024]
            xah, xal = _limbs2(xa)
            xbh, xbl = _limbs2(xb)
            xm[0, t * 512 : (t + 1) * 512] = xah.astype(np.float64)
            xm[1, t * 512 : (t + 1) * 512] = xal.astype(np.float64)
            xm[2, t * 512 : (t + 1) * 512] = xbh.astype(np.float64)
            xm[3, t * 512 : (t + 1) * 512] = xbl.astype(np.float64)
        xm[4, : NT * 512] = 1.0
        xm[5, : NT * 512] = 0.0
        xm[:, NT * 512 :] = wu
        in_maps.append(
            {
                "klb": np.ascontiguousarray(klb_all[c].astype(ml_dtypes.bfloat16)),
                "krb": np.ascontiguousarray(krb_all[c].astype(ml_dtypes.bfloat16)),
                "xmw": np.ascontiguousarray(xm.astype(ml_dtypes.bfloat16)),
                "wpc": np.ascontiguousarray(wp),
            }
        )
    return in_maps


def kernel(x_input, targets, w1, b1, w2, b2, **_ignored):
    in_maps = make_in_maps(x_input, targets, w1, b1, w2, b2)
    nc = _get_nc()
    res = run_bass_kernel_spmd(nc, in_maps, core_ids=list(range(NCORES)))

    # ---- host epilogue: density from per-element top-2 non-self -d^2 ----
    targets64 = np.asarray(targets, np.float64)
    b20 = float(np.asarray(b2).ravel()[0])
    dsum = []
    sse = 0.0
    d2sq = 0.0
    for c, r in enumerate(res.results):
        t = r["td"].astype(np.float64).reshape(P, TPC, 8)
        tt = t[:, :, 1:3]  # 2nd/3rd largest = the two nearest (non-self)
        d = np.sqrt(np.maximum(-tt, 0.0))
        dsum.append(d.sum(axis=2).ravel())
        pr = r["po"].astype(np.float64)  # rows 0:4 pred, 4:8 d2
        tsh = targets64[c * SHARD : (c + 1) * SHARD]
        for tt_ in range(NT):
            e_a = pr[2 * tt_] + b20 - tsh[tt_ * 1024 : tt_ * 1024 + 512]
            e_b = pr[2 * tt_ + 1] + b20 - tsh[tt_ * 1024 + 512 : (tt_ + 1) * 1024]
            sse += (e_a**2).sum() + (e_b**2).sum()
        d2sq += (pr[4:8] ** 2).sum()
    d12 = np.concatenate(dsum)
    dens = 1.0 / (d12 / 3.0 + 2.0 * EPS)
    m = (dens.sum() / N) / (dens.max() + EPS)

    mse = sse / N
    penalty = 0.01 * (1.0 + 0.1 * m) * (d2sq / N)
    total = mse + penalty
    return np.array([total, mse, penalty], dtype=np.float32)


# revision 53
# speedup vs baseline: 1.1678x; 1.0144x over previous
"""AdaptiveCurvatureLoss on 8 TRN2 NeuronCores — bucketed exact kNN via
matmul + DVE top-8, no sort.

The reference needs, per element, the mean of the 3 smallest |x_i - x_j|
(incl. self-zero), then only mean(dens) and max(dens).  Host shards the
N=16384 samples by VALUE into 128 quantile buckets of exactly 128 elements
(one np.partition call — the sharding step), 16 buckets per core.  The two
nearest neighbours of any element provably lie inside its bucket plus a
2-element halo on each side, so each core evaluates the reference's NxN
pairwise matrix restricted to 16 row-blocks of [128 x 132]:

  -d^2[i,j] = -(x_i - x_j)^2  via one K=10 bf16 matmul per block
              (two-limb bf16 split of x and three-limb split of x^2 keeps
              products exact in fp32 PSUM; -d^2 error ~1e-10, self == ~0),
  top-3 nearest = DVE max8 (top-8 per partition) straight out of PSUM.

The MLP / second-derivative / MSE parts run in the transposed layout
(hidden units on partitions, 512 points per tile, two 64-wide h-blocks per
128 partitions): u = w1*x + b1 as one K=6 two-limb bf16 matmul, tanh /
square on ACT, g = (1-th^2)*th on DVE, then bf16 PE matmuls compute
e = pred + b2 - t and d2 in a single [8,512] PSUM tile; one ACT
Square+accum yields the 8 scalar partials.

Host epilogue (O(N) numpy): d1+d2 = sqrt of the top-2 non-self -d^2,
density mean/max, final three scalars.
"""

import sys

sys.path.insert(0, "/opt/trn_rl_repo")

import numpy as np

import concourse.mybir as mybir
from concourse import bacc
from concourse.bass_utils import run_bass_kernel_spmd
from concourse.tile import TileContext

N = 16384
NCORES = 8
SHARD = N // NCORES          # 2048
P = 128
NB = N // P                  # 128 value buckets of 128 elements
TPC = NB // NCORES           # 16 buckets (tiles) per core
C = P + 4                    # candidate columns: bucket + 2-elem halo each side
KK = 10                      # K rows of the kNN matmul
EPS = 1e-8
SENT = 1e8                   # sentinel -d^2 magnitude for missing halo
F32 = mybir.dt.float32
BF16 = mybir.dt.bfloat16
ALU = mybir.AluOpType
ACTF = mybir.ActivationFunctionType

NT = 2                       # packed [128, 512] MLP tiles per core (2048 pts)


def _build():
    nc = bacc.Bacc()
    klb = nc.declare_dram_parameter("klb", [KK, TPC * P], BF16, isOutput=False)
    krb = nc.declare_dram_parameter("krb", [KK, TPC * C], BF16, isOutput=False)
    xmw = nc.declare_dram_parameter("xmw", [6, NT * 512 + P], BF16, isOutput=False)
    wpc = nc.declare_dram_parameter("wpc", [P, 4 * 8], BF16, isOutput=False)
    td = nc.declare_dram_parameter("td", [P, TPC * 8], F32, isOutput=True)
    po = nc.declare_dram_parameter("po", [8, 512], F32, isOutput=True)

    with TileContext(nc) as tc:
        with (
            tc.tile_pool(name="sp", bufs=1) as sp,
            tc.tile_pool(name="kp", bufs=4, space="PSUM") as kp,
            tc.tile_pool(name="mu", bufs=2, space="PSUM") as mu,
            tc.tile_pool(name="ms", bufs=2, space="PSUM") as ms,
        ):
            # loads: xmw first on sync (heads the serial MLP chain), kNN
            # packs split so tile-0 inputs land earliest
            HP, HC = (TPC // 2) * P, (TPC // 2) * C
            klbt = sp.tile([KK, TPC * P], BF16)
            krbt = sp.tile([KK, TPC * C], BF16)
            xmt = sp.tile([6, NT * 512 + P], BF16)
            nc.sync.dma_start(xmt[:, :], xmw[:, :])
            nc.scalar.dma_start(klbt[:, 0:HP], klb[:, 0:HP])
            nc.sync.dma_start(krbt[:, 0:HC], krb[:, 0:HC])
            nc.scalar.dma_start(krbt[:, HC:], krb[:, HC:])
            nc.gpsimd.dma_start(klbt[:, HP:], klb[:, HP:])
            wpt = sp.tile([P, 4 * 8], BF16)
            nc.gpsimd.dma_start(wpt[:, :], wpc[:, :])

            th = sp.tile([P, NT, 512], BF16)
            th2 = sp.tile([P, NT, 512], BF16)
            th3 = sp.tile([P, NT, 512], BF16)
            tds = sp.tile([P, TPC * 8], F32)
            posb = sp.tile([8, 512], F32)

            ps8 = ms.tile([8, 512], F32, tag="ms")

            # ---- emitters ----
            def emit_knn(t):
                ps = kp.tile([P, C], F32, tag="kp")
                nc.tensor.matmul(
                    ps[:, :], klbt[:, t * P : (t + 1) * P],
                    krbt[:, t * C : (t + 1) * C], start=True, stop=True,
                )
                # bias the scheduler: let the MLP's th3 TT slot in before the
                # late max8s instead of after all of them
                with tc.tile_wait_until(0.013, enable=(t >= 8)):
                    nc.vector.max(tds[:, 8 * t : 8 * t + 8], ps[:, :])

            def emit_u_tanh(t):
                ups = mu.tile([P, 512], F32, tag="mu")
                nc.tensor.matmul(
                    ups[:, :], xmt[:, NT * 512 : NT * 512 + P],
                    xmt[:, t * 512 : (t + 1) * 512], start=True, stop=True,
                )
                nc.scalar.activation(th[:, t, :], ups[:, :], ACTF.Tanh)

            def emit_sq_all():
                nc.scalar.activation(
                    th2[:, :, :].rearrange("p a b -> p (a b)"),
                    th[:, :, :].rearrange("p a b -> p (a b)"), ACTF.Square,
                )

            def emit_th3_all():
                # th^3 = th2 * th: plain bf16 TT -> DVE 2x_1P, one op both tiles
                nc.vector.tensor_tensor(
                    th3[:, :, :].rearrange("p a b -> p (a b)"),
                    th2[:, :, :].rearrange("p a b -> p (a b)"),
                    th[:, :, :].rearrange("p a b -> p (a b)"),
                    op=ALU.mult,
                )

            def emit_mlp_a(t):
                # one matmul: psum rows 0:4 get pred (w2), rows 4:8 get
                # -c2n*th, via an 8-col zero-padded lhsT
                nc.tensor.matmul(
                    ps8[:, :], wpt[:, 8 * t : 8 * t + 8], th[:, t, :],
                    start=(t == 0), stop=False,
                )

            def emit_mlp_b(t):
                # rows 4:8 += c2n*th3 (cols 0:4 of the pack are zero)
                nc.tensor.matmul(
                    ps8[:, :], wpt[:, 16 + 8 * t : 24 + 8 * t], th3[:, t, :],
                    start=False, stop=(t == NT - 1),
                )

            def emit_po_out():
                # pred and d2 out; host does targets/squares
                nc.scalar.activation(posb[:, :], ps8[:, :], ACTF.Copy)
                nc.gpsimd.dma_start(po[:, :], posb[:, :])

            # ---- schedule: MLP chain heads the queues (high priority so the
            # scheduler never parks its ops behind the kNN stream) ----
            with tc.high_priority():
                emit_u_tanh(0)
                emit_u_tanh(1)
            for t in range(TPC):
                emit_knn(t)
                if t == 0:
                    with tc.high_priority():
                        emit_sq_all()
                elif t == 1:
                    with tc.high_priority():
                        emit_mlp_a(0)
                        emit_mlp_a(1)
                elif t == 2:
                    with tc.high_priority():
                        emit_th3_all()
                elif t == 3:
                    with tc.high_priority():
                        emit_mlp_b(0)
                        emit_mlp_b(1)
                elif t == 4:
                    with tc.high_priority():
                        emit_po_out()
                if t % 4 == 3 and t < TPC - 1:
                    q = nc.sync if (t // 4) % 2 == 0 else nc.gpsimd
                    q.dma_start(
                        td[:, 8 * (t - 3) : 8 * (t + 1)],
                        tds[:, 8 * (t - 3) : 8 * (t + 1)],
                    )
            nc.gpsimd.dma_start(td[:, 96:120], tds[:, 96:120])
            nc.sync.dma_start(td[:, 120:128], tds[:, 120:128])
    nc.finalize()
    return nc


_NC_CACHE = None


def _get_nc():
    global _NC_CACHE
    if _NC_CACHE is None:
        _NC_CACHE = _build()
    return _NC_CACHE


def _b16(a):
    import ml_dtypes

    return np.asarray(a, dtype=np.float64).astype(ml_dtypes.bfloat16)


def _limbs2(v):
    """Split f64 array into 2 bf16 limbs (value approx h+l)."""
    h = _b16(v)
    l = _b16(np.asarray(v, np.float64) - h.astype(np.float64))
    return h, l


def _limbs3(v):
    h = _b16(v)
    r = np.asarray(v, np.float64) - h.astype(np.float64)
    m = _b16(r)
    lo = _b16(r - m.astype(np.float64))
    return h, m, lo


def make_in_maps(x_input, targets, w1, b1, w2, b2):
    import ml_dtypes

    x_input = np.ascontiguousarray(x_input, dtype=np.float32)
    targets = np.ascontiguousarray(targets, dtype=np.float32)
    w1 = np.asarray(w1, dtype=np.float32)
    b1 = np.asarray(b1, dtype=np.float32)
    w2 = np.asarray(w2, dtype=np.float32)
    b2 = np.asarray(b2, dtype=np.float32)

    # ---- value-bucket sharding: 128 buckets of exactly 128 + halo stats ----
    kth = np.unique(
        np.concatenate(
            [np.arange(1, NB) * P + d for d in (-2, -1, 0, 1)]
        )
    )
    part = np.partition(x_input, kth).astype(np.float64)

    klb_all = []
    krb_all = []
    for c in range(NCORES):
        klc = np.zeros((KK, TPC * P), np.float64)
        krc = np.zeros((KK, TPC * C), np.float64)
        krc[0:3, :] = -1.0
        for ti in range(TPC):
            b = c * TPC + ti
            blk = part[b * P : (b + 1) * P]
            lo = part[b * P - 2 : b * P] if b > 0 else None
            hi = part[(b + 1) * P : (b + 1) * P + 2] if b < NB - 1 else None
            ctr = np.float32((blk.min() + blk.max()) / 2.0)

            xi = (blk - ctr).astype(np.float32).astype(np.float64)
            xih, xil = _limbs2(xi)
            xi_hat = xih.astype(np.float64) + xil.astype(np.float64)
            si = xi_hat**2
            sih, sim, sil = _limbs3(si)
            one = np.ones(P)
            klc[:, ti * P : (ti + 1) * P] = np.stack(
                [
                    sih.astype(np.float64), sim.astype(np.float64),
                    sil.astype(np.float64),
                    xih.astype(np.float64), xih.astype(np.float64),
                    xil.astype(np.float64), xil.astype(np.float64),
                    one, one, one,
                ]
            )

            # candidates: [lo2 | bucket | hi2]
            cvals = np.zeros(C, np.float64)
            creal = np.ones(C, bool)
            cvals[2 : 2 + P] = blk
            if lo is not None:
                cvals[0:2] = lo
            else:
                creal[0:2] = False
            if hi is not None:
                cvals[2 + P :] = hi
            else:
                creal[2 + P :] = False
            xj = (cvals - ctr).astype(np.float32).astype(np.float64)
            xjh, xjl = _limbs2(xj)
            xj_hat = xjh.astype(np.float64) + xjl.astype(np.float64)
            sj = xj_hat**2
            sjh, sjm, sjl = _limbs3(sj)
            xjh64 = xjh.astype(np.float64)
            xjl64 = xjl.astype(np.float64)
            kr = np.stack(
                [
                    2 * xjh64, 2 * xjl64, 2 * xjh64, 2 * xjl64,
                    -sjh.astype(np.float64), -sjm.astype(np.float64),
                    -sjl.astype(np.float64),
                ]
            )
            # sentinel columns: x-limbs 0, s_hi = SENT -> -d^2 ~= -SENT
            bad = ~creal
            kr[0:4, bad] = 0.0
            kr[4, bad] = -SENT
            kr[5:7, bad] = 0.0
            krc[3:10, ti * C : (ti + 1) * C] = kr
        klb_all.append(klc)
        krb_all.append(krc)

    # ---- MLP packs (two-limb u-matmul, rest as in the sort baseline) ----
    w1h, w1l = _limbs2(w1)
    b1h, b1l = _limbs2(b1)
    H = 64
    wu = np.zeros((6, P), np.float64)
    wu[0, :H] = w1h.astype(np.float64)
    wu[1, :H] = w1l.astype(np.float64)
    wu[2, H:] = w1h.astype(np.float64)
    wu[3, H:] = w1l.astype(np.float64)
    wu[4, :H] = b1h.astype(np.float64)
    wu[4, H:] = b1h.astype(np.float64)
    wu[5, :H] = b1l.astype(np.float64)
    wu[5, H:] = b1l.astype(np.float64)

    c2n = (2.0 * w1.astype(np.float64) ** 2 * w2.astype(np.float64)).astype(
        np.float32
    )
    # 8-col zero-padded lhsT packs: A(t) = [pred | -c2n*th], B(t) = [0 | +c2n*th3]
    wp = np.zeros((P, 32), np.float32)
    for t in range(NT):
        a0 = 8 * t
        wp[:H, a0 + 2 * t] = w2
        wp[H:, a0 + 2 * t + 1] = w2
        wp[:H, a0 + 4 + 2 * t] = -c2n
        wp[H:, a0 + 4 + 2 * t + 1] = -c2n
        b0 = 16 + 8 * t
        wp[:H, b0 + 4 + 2 * t] = c2n
        wp[H:, b0 + 4 + 2 * t + 1] = c2n
    wp = wp.astype(ml_dtypes.bfloat16)

    in_maps = []
    for c in range(NCORES):
        xsh = x_input[c * SHARD : (c + 1) * SHARD].astype(np.float64)
        xm = np.zeros((6, NT * 512 + P), np.float64)
        for t in range(NT):
            xa = xsh[t * 1024 : t * 1024 + 512]
            xb = xsh[t * 1024 + 512 : (t + 1) * 1024]
            xah, xal = _limbs2(xa)
            xbh, xbl = _limbs2(xb)
            xm[0, t * 512 : (t + 1) * 512] = xah.astype(np.float64)
            xm[1, t * 512 : (t + 1) * 512] = xal.astype(np.float64)
            xm[2, t * 512 : (t + 1) * 512] = xbh.astype(np.float64)
            xm[3, t * 512 : (t + 1) * 512] = xbl.astype(np.float64)
        xm[4, : NT * 512] = 1.0
        xm[5, : NT * 512] = 0.0
        xm[:, NT * 512 :] = wu
        in_maps.append(
            {
                "klb": np.ascontiguousarray(klb_all[c].astype(ml_dtypes.bfloat16)),
                "krb": np.ascontiguousarray(krb_all[c].astype(ml_dtypes.bfloat16)),
                "xmw": np.ascontiguousarray(xm.astype(ml_dtypes.bfloat16)),
                "wpc": np.ascontiguousarray(wp),
            }
        )
    return in_maps


def kernel(x_input, targets, w1, b1, w2, b2, **_ignored):
    in_maps = make_in_maps(x_input, targets, w1, b1, w2, b2)
    nc = _get_nc()
    res = run_bass_kernel_spmd(nc, in_maps, core_ids=list(range(NCORES)))

    # ---- host epilogue: density from per-element top-2 non-self -d^2 ----
    targets64 = np.asarray(targets, np.float64)
    b20 = float(np.asarray(b2).ravel()[0])
    dsum = []
    sse = 0.0
    d2sq = 0.0
    for c, r in enumerate(res.results):
        t = r["td"].astype(np.float64).reshape(P, TPC, 8)
        tt = t[:, :, 1:3]  # 2nd/3rd largest = the two nearest (non-self)
        d = np.sqrt(np.maximum(-tt, 0.0))
        dsum.append(d.sum(axis=2).ravel())
        pr = r["po"].astype(np.float64)  # rows 0:4 pred, 4:8 d2
        tsh = targets64[c * SHARD : (c + 1) * SHARD]
        for tt_ in range(NT):
            e_a = pr[2 * tt_] + b20 - tsh[tt_ * 1024 : tt_ * 1024 + 512]
            e_b = pr[2 * tt_ + 1] + b20 - tsh[tt_ * 1024 + 512 : (tt_ + 1) * 1024]
            sse += (e_a**2).sum() + (e_b**2).sum()
        d2sq += (pr[4:8] ** 2).sum()
    d12 = np.concatenate(dsum)
    dens = 1.0 / (d12 / 3.0 + 2.0 * EPS)
    m = (dens.sum() / N) / (dens.max() + EPS)

    mse = sse / N
    penalty = 0.01 * (1.0 + 0.1 * m) * (d2sq / N)
    total = mse + penalty
    return np.array([total, mse, penalty], dtype=np.float32)


# revision 55
# speedup vs baseline: 1.1778x; 1.0085x over previous
"""AdaptiveCurvatureLoss on 8 TRN2 NeuronCores — bucketed exact kNN via
matmul + DVE top-8, no sort.

The reference needs, per element, the mean of the 3 smallest |x_i - x_j|
(incl. self-zero), then only mean(dens) and max(dens).  Host shards the
N=16384 samples by VALUE into 128 quantile buckets of exactly 128 elements
(one np.partition call — the sharding step), 16 buckets per core.  The two
nearest neighbours of any element provably lie inside its bucket plus a
2-element halo on each side, so each core evaluates the reference's NxN
pairwise matrix restricted to 16 row-blocks of [128 x 132]:

  -d^2[i,j] = -(x_i - x_j)^2  via one K=10 bf16 matmul per block
              (two-limb bf16 split of x and three-limb split of x^2 keeps
              products exact in fp32 PSUM; -d^2 error ~1e-10, self == ~0),
  top-3 nearest = DVE max8 (top-8 per partition) straight out of PSUM.

The MLP / second-derivative / MSE parts run in the transposed layout
(hidden units on partitions, 512 points per tile, two 64-wide h-blocks per
128 partitions): u = w1*x + b1 as one K=6 two-limb bf16 matmul, tanh /
square on ACT, g = (1-th^2)*th on DVE, then bf16 PE matmuls compute
e = pred + b2 - t and d2 in a single [8,512] PSUM tile; one ACT
Square+accum yields the 8 scalar partials.

Host epilogue (O(N) numpy): d1+d2 = sqrt of the top-2 non-self -d^2,
density mean/max, final three scalars.
"""

import sys

sys.path.insert(0, "/opt/trn_rl_repo")

import numpy as np

import concourse.mybir as mybir
from concourse import bacc
from concourse.bass_utils import run_bass_kernel_spmd
from concourse.tile import TileContext

N = 16384
NCORES = 8
SHARD = N // NCORES          # 2048
P = 128
NB = N // P                  # 128 value buckets of 128 elements
TPC = NB // NCORES           # 16 buckets (tiles) per core
C = P + 4                    # candidate columns: bucket + 2-elem halo each side
KK = 10                      # K rows of the kNN matmul
EPS = 1e-8
SENT = 1e8                   # sentinel -d^2 magnitude for missing halo
F32 = mybir.dt.float32
BF16 = mybir.dt.bfloat16
ALU = mybir.AluOpType
ACTF = mybir.ActivationFunctionType

NT = 2                       # packed [128, 512] MLP tiles per core (2048 pts)


def _build():
    nc = bacc.Bacc()
    klb = nc.declare_dram_parameter("klb", [KK, TPC * P], BF16, isOutput=False)
    krb = nc.declare_dram_parameter("krb", [KK, TPC * C], BF16, isOutput=False)
    xmw = nc.declare_dram_parameter("xmw", [6, NT * 512 + P], BF16, isOutput=False)
    wpc = nc.declare_dram_parameter("wpc", [P, 4 * 8], BF16, isOutput=False)
    td = nc.declare_dram_parameter("td", [P, TPC * 8], F32, isOutput=True)
    po = nc.declare_dram_parameter("po", [8, 512], F32, isOutput=True)

    with TileContext(nc) as tc:
        with (
            tc.tile_pool(name="sp", bufs=1) as sp,
            tc.tile_pool(name="kp", bufs=5, space="PSUM") as kp,
            tc.tile_pool(name="mu", bufs=2, space="PSUM") as mu,
            tc.tile_pool(name="ms", bufs=1, space="PSUM") as ms,
        ):
            # loads: xmw first on sync (heads the serial MLP chain), kNN
            # packs split so tile-0 inputs land earliest
            HP, HC = (TPC // 2) * P, (TPC // 2) * C
            klbt = sp.tile([KK, TPC * P], BF16)
            krbt = sp.tile([KK, TPC * C], BF16)
            xmt = sp.tile([6, NT * 512 + P], BF16)
            nc.sync.dma_start(xmt[:, :], xmw[:, :])
            nc.gpsimd.dma_start(klbt[:, 0:HP], klb[:, 0:HP])
            nc.sync.dma_start(krbt[:, 0:HC], krb[:, 0:HC])
            nc.scalar.dma_start(klbt[:, HP:], klb[:, HP:])
            nc.scalar.dma_start(krbt[:, HC:], krb[:, HC:])
            wpt = sp.tile([P, 4 * 8], BF16)
            nc.gpsimd.dma_start(wpt[:, :], wpc[:, :])

            th = sp.tile([P, NT, 512], BF16)
            th2 = sp.tile([P, NT, 512], BF16)
            th3 = sp.tile([P, NT, 512], BF16)
            tds = sp.tile([P, TPC * 8], F32)
            posb = sp.tile([8, 512], F32)

            ps8 = ms.tile([8, 512], F32, tag="ms")

            # ---- emitters ----
            def emit_knn(t):
                ps = kp.tile([P, C], F32, tag="kp")
                nc.tensor.matmul(
                    ps[:, :], klbt[:, t * P : (t + 1) * P],
                    krbt[:, t * C : (t + 1) * C], start=True, stop=True,
                )
                # bias the scheduler: let the MLP's th3 TT slot in before the
                # late max8s instead of after all of them
                with tc.tile_wait_until(0.013, enable=(t >= 8)):
                    nc.vector.max(tds[:, 8 * t : 8 * t + 8], ps[:, :])

            def emit_u_tanh(t):
                ups = mu.tile([P, 512], F32, tag="mu")
                nc.tensor.matmul(
                    ups[:, :], xmt[:, NT * 512 : NT * 512 + P],
                    xmt[:, t * 512 : (t + 1) * 512], start=True, stop=True,
                )
                nc.scalar.activation(th[:, t, :], ups[:, :], ACTF.Tanh)

            def emit_sq_all():
                nc.scalar.activation(
                    th2[:, :, :].rearrange("p a b -> p (a b)"),
                    th[:, :, :].rearrange("p a b -> p (a b)"), ACTF.Square,
                )

            def emit_th3_all():
                # th^3 = th2 * th: plain bf16 TT -> DVE 2x_1P, one op both tiles
                nc.vector.tensor_tensor(
                    th3[:, :, :].rearrange("p a b -> p (a b)"),
                    th2[:, :, :].rearrange("p a b -> p (a b)"),
                    th[:, :, :].rearrange("p a b -> p (a b)"),
                    op=ALU.mult,
                )

            def emit_mlp_a(t):
                # one matmul: psum rows 0:4 get pred (w2), rows 4:8 get
                # -c2n*th, via an 8-col zero-padded lhsT
                nc.tensor.matmul(
                    ps8[:, :], wpt[:, 8 * t : 8 * t + 8], th[:, t, :],
                    start=(t == 0), stop=False,
                )

            def emit_mlp_b(t):
                # rows 4:8 += c2n*th3 (cols 0:4 of the pack are zero)
                nc.tensor.matmul(
                    ps8[:, :], wpt[:, 16 + 8 * t : 24 + 8 * t], th3[:, t, :],
                    start=False, stop=(t == NT - 1),
                )

            def emit_po_out():
                # pred and d2 out; host does targets/squares
                nc.scalar.activation(posb[:, :], ps8[:, :], ACTF.Copy)
                nc.gpsimd.dma_start(po[:, :], posb[:, :])

            # ---- schedule: MLP chain heads the queues (high priority so the
            # scheduler never parks its ops behind the kNN stream) ----
            with tc.high_priority():
                emit_u_tanh(0)
                emit_u_tanh(1)
            for t in range(TPC):
                emit_knn(t)
                if t == 0:
                    with tc.high_priority():
                        emit_sq_all()
                elif t == 1:
                    with tc.high_priority():
                        emit_mlp_a(0)
                        emit_mlp_a(1)
                elif t == 2:
                    with tc.high_priority():
                        emit_th3_all()
                elif t == 3:
                    with tc.high_priority():
                        emit_mlp_b(0)
                        emit_mlp_b(1)
                elif t == 4:
                    with tc.high_priority():
                        emit_po_out()
                if t % 4 == 3 and t < TPC - 1:
                    q = nc.sync if (t // 4) % 2 == 0 else nc.gpsimd
                    q.dma_start(
                        td[:, 8 * (t - 3) : 8 * (t + 1)],
                        tds[:, 8 * (t - 3) : 8 * (t + 1)],
                    )
            nc.gpsimd.dma_start(td[:, 96:120], tds[:, 96:120])
            nc.sync.dma_start(td[:, 120:128], tds[:, 120:128])
    nc.finalize()
    return nc


_NC_CACHE = None


def _get_nc():
    global _NC_CACHE
    if _NC_CACHE is None:
        _NC_CACHE = _build()
    return _NC_CACHE


def _b16(a):
    import ml_dtypes

    return np.asarray(a, dtype=np.float64).astype(ml_dtypes.bfloat16)


def _limbs2(v):
    """Split f64 array into 2 bf16 limbs (value approx h+l)."""
    h = _b16(v)
    l = _b16(np.asarray(v, np.float64) - h.astype(np.float64))
    return h, l


def _limbs3(v):
    h = _b16(v)
    r = np.asarray(v, np.float64) - h.astype(np.float64)
    m = _b16(r)
    lo = _b16(r - m.astype(np.float64))
    return h, m, lo


def make_in_maps(x_input, targets, w1, b1, w2, b2):
    import ml_dtypes

    x_input = np.ascontiguousarray(x_input, dtype=np.float32)
    targets = np.ascontiguousarray(targets, dtype=np.float32)
    w1 = np.asarray(w1, dtype=np.float32)
    b1 = np.asarray(b1, dtype=np.float32)
    w2 = np.asarray(w2, dtype=np.float32)
    b2 = np.asarray(b2, dtype=np.float32)

    # ---- value-bucket sharding: 128 buckets of exactly 128 + halo stats ----
    kth = np.unique(
        np.concatenate(
            [np.arange(1, NB) * P + d for d in (-2, -1, 0, 1)]
        )
    )
    part = np.partition(x_input, kth).astype(np.float64)

    klb_all = []
    krb_all = []
    for c in range(NCORES):
        klc = np.zeros((KK, TPC * P), np.float64)
        krc = np.zeros((KK, TPC * C), np.float64)
        krc[0:3, :] = -1.0
        for ti in range(TPC):
            b = c * TPC + ti
            blk = part[b * P : (b + 1) * P]
            lo = part[b * P - 2 : b * P] if b > 0 else None
            hi = part[(b + 1) * P : (b + 1) * P + 2] if b < NB - 1 else None
            ctr = np.float32((blk.min() + blk.max()) / 2.0)

            xi = (blk - ctr).astype(np.float32).astype(np.float64)
            xih, xil = _limbs2(xi)
            xi_hat = xih.astype(np.float64) + xil.astype(np.float64)
            si = xi_hat**2
            sih, sim, sil = _limbs3(si)
            one = np.ones(P)
            klc[:, ti * P : (ti + 1) * P] = np.stack(
                [
                    sih.astype(np.float64), sim.astype(np.float64),
                    sil.astype(np.float64),
                    xih.astype(np.float64), xih.astype(np.float64),
                    xil.astype(np.float64), xil.astype(np.float64),
                    one, one, one,
                ]
            )

            # candidates: [lo2 | bucket | hi2]
            cvals = np.zeros(C, np.float64)
            creal = np.ones(C, bool)
            cvals[2 : 2 + P] = blk
            if lo is not None:
                cvals[0:2] = lo
            else:
                creal[0:2] = False
            if hi is not None:
                cvals[2 + P :] = hi
            else:
                creal[2 + P :] = False
            xj = (cvals - ctr).astype(np.float32).astype(np.float64)
            xjh, xjl = _limbs2(xj)
            xj_hat = xjh.astype(np.float64) + xjl.astype(np.float64)
            sj = xj_hat**2
            sjh, sjm, sjl = _limbs3(sj)
            xjh64 = xjh.astype(np.float64)
            xjl64 = xjl.astype(np.float64)
            kr = np.stack(
                [
                    2 * xjh64, 2 * xjl64, 2 * xjh64, 2 * xjl64,
                    -sjh.astype(np.float64), -sjm.astype(np.float64),
                    -sjl.astype(np.float64),
                ]
            )
            # sentinel columns: x-limbs 0, s_hi = SENT -> -d^2 ~= -SENT
            bad = ~creal
            kr[0:4, bad] = 0.0
            kr[4, bad] = -SENT
            kr[5:7, bad] = 0.0
            krc[3:10, ti * C : (ti + 1) * C] = kr
        klb_all.append(klc)
        krb_all.append(krc)

    # ---- MLP packs (two-limb u-matmul, rest as in the sort baseline) ----
    w1h, w1l = _limbs2(w1)
    b1h, b1l = _limbs2(b1)
    H = 64
    wu = np.zeros((6, P), np.float64)
    wu[0, :H] = w1h.astype(np.float64)
    wu[1, :H] = w1l.astype(np.float64)
    wu[2, H:] = w1h.astype(np.float64)
    wu[3, H:] = w1l.astype(np.float64)
    wu[4, :H] = b1h.astype(np.float64)
    wu[4, H:] = b1h.astype(np.float64)
    wu[5, :H] = b1l.astype(np.float64)
    wu[5, H:] = b1l.astype(np.float64)

    c2n = (2.0 * w1.astype(np.float64) ** 2 * w2.astype(np.float64)).astype(
        np.float32
    )
    # 8-col zero-padded lhsT packs: A(t) = [pred | -c2n*th], B(t) = [0 | +c2n*th3]
    wp = np.zeros((P, 32), np.float32)
    for t in range(NT):
        a0 = 8 * t
        wp[:H, a0 + 2 * t] = w2
        wp[H:, a0 + 2 * t + 1] = w2
        wp[:H, a0 + 4 + 2 * t] = -c2n
        wp[H:, a0 + 4 + 2 * t + 1] = -c2n
        b0 = 16 + 8 * t
        wp[:H, b0 + 4 + 2 * t] = c2n
        wp[H:, b0 + 4 + 2 * t + 1] = c2n
    wp = wp.astype(ml_dtypes.bfloat16)

    in_maps = []
    for c in range(NCORES):
        xsh = x_input[c * SHARD : (c + 1) * SHARD].astype(np.float64)
        xm = np.zeros((6, NT * 512 + P), np.float64)
        for t in range(NT):
            xa = xsh[t * 1024 : t * 1024 + 512]
            xb = xsh[t * 1024 + 512 : (t + 1) * 1024]
            xah, xal = _limbs2(xa)
            xbh, xbl = _limbs2(xb)
            xm[0, t * 512 : (t + 1) * 512] = xah.astype(np.float64)
            xm[1, t * 512 : (t + 1) * 512] = xal.astype(np.float64)
            xm[2, t * 512 : (t + 1) * 512] = xbh.astype(np.float64)
            xm[3, t * 512 : (t + 1) * 512] = xbl.astype(np.float64)
        xm[4, : NT * 512] = 1.0
        xm[5, : NT * 512] = 0.0
        xm[:, NT * 512 :] = wu
        in_maps.append(
            {
                "klb": np.ascontiguousarray(klb_all[c].astype(ml_dtypes.bfloat16)),
                "krb": np.ascontiguousarray(krb_all[c].astype(ml_dtypes.bfloat16)),
                "xmw": np.ascontiguousarray(xm.astype(ml_dtypes.bfloat16)),
                "wpc": np.ascontiguousarray(wp),
            }
        )
    return in_maps


def kernel(x_input, targets, w1, b1, w2, b2, **_ignored):
    in_maps = make_in_maps(x_input, targets, w1, b1, w2, b2)
    nc = _get_nc()
    res = run_bass_kernel_spmd(nc, in_maps, core_ids=list(range(NCORES)))

    # ---- host epilogue: density from per-element top-2 non-self -d^2 ----
    targets64 = np.asarray(targets, np.float64)
    b20 = float(np.asarray(b2).ravel()[0])
    dsum = []
    sse = 0.0
    d2sq = 0.0
    for c, r in enumerate(res.results):
        t = r["td"].astype(np.float64).reshape(P, TPC, 8)
        tt = t[:, :, 1:3]  # 2nd/3rd largest = the two nearest (non-self)
        d = np.sqrt(np.maximum(-tt, 0.0))
        dsum.append(d.sum(axis=2).ravel())
        pr = r["po"].astype(np.float64)  # rows 0:4 pred, 4:8 d2
        tsh = targets64[c * SHARD : (c + 1) * SHARD]
        for tt_ in range(NT):
            e_a = pr[2 * tt_] + b20 - tsh[tt_ * 1024 : tt_ * 1024 + 512]
            e_b = pr[2 * tt_ + 1] + b20 - tsh[tt_ * 1024 + 512 : (tt_ + 1) * 1024]
            sse += (e_a**2).sum() + (e_b**2).sum()
        d2sq += (pr[4:8] ** 2).sum()
    d12 = np.concatenate(dsum)
    dens = 1.0 / (d12 / 3.0 + 2.0 * EPS)
    m = (dens.sum() / N) / (dens.max() + EPS)

    mse = sse / N
    penalty = 0.01 * (1.0 + 0.1 * m) * (d2sq / N)
    total = mse + penalty
    return np.array([total, mse, penalty], dtype=np.float32)
